# revision 34
# baseline (speedup 1.0000x reference)
"""Trainium2 Bass kernel: float32 -> 32-channel bit-plane encoding.

For input x [4096, 512] f32, produces out [4096, 512, 32] f32 where
out[b, f, 0] = (x[b,f] < 0) and out[b, f, 1+j] = bit (30-j) of
bitcast_int32(|x[b,f]|), MSB first.

Host-side repack makes the device work a pure byte-lane bit extraction:
  i' = (bitcast_i32(x) & 0x7FFFFFFF) | ((x < 0) << 31)
so channel k of feature f is bit (31-k) of i'[f].  The four bytes of i'
are de-interleaved into blocks (MSB block first):
  xb[r] = [byte3(f) for f] ++ [byte2(f)] ++ [byte1(f)] ++ [byte0(f)]
and viewed as u16 words (feature pairs 2j, 2j+1 inside each block).

Device (per 128-row tile, per shift s in 0..7), one DVE instruction:
  out_u16 = (w >> s) & 0x0101
which extracts bit s of BOTH packed bytes into the two u8 lanes of the
word — i.e. one instruction emits the {0,1} bytes of four whole channel
planes (channels 8g + 7-s for blocks g=0..3).  All operands are 2-byte
so the DVE runs in its fast packed mode; device output is u8 planes
(8.4 MB/core instead of 32 MB/core f32 -> ~4x less HBM write traffic,
which is the bottleneck in this memory-bound problem).

DRAM out layout per row: [s(8), g(4), f(512)] u8.  The host unshard step
restores [f, k] order (k = 8g + (7-s)) and widens {0,1} u8 -> f32, which
is exact.

Sharded row-wise over 8 NeuronCores (512 rows each).  Per core:
  in-DMA  (HWDGE):   4 row tiles x 256 KB u16; tile 0 row-split across
                     the sync+scalar rings so it lands earliest
  compute (VectorE): 32 tensor_scalar instructions (~335 ns each)
  out-DMA (sync):    u8 planes; small leading pieces, then 2 MB pieces
"""

import sys

if "/opt/trn_rl_repo" not in sys.path:
    sys.path.insert(0, "/opt/trn_rl_repo")

import numpy as np

import concourse.bass as bass
import concourse.mybir as mybir

P = 128            # SBUF partitions
F = 512            # features per row
K = 32             # output channels per feature
NS = 8             # bit shifts per byte
NG = 4             # byte blocks per word
N_CORES = 8
ROWS_TOTAL = 4096
ROWS = ROWS_TOTAL // N_CORES   # rows per core
NRT = ROWS // P                # row tiles per core (4)
W = F * NG // 2                # u16 words per row (1024)
OB = F * K                     # out bytes per row (16384)

# Out-DMA pieces as (rt, lo_byte, hi_byte, ts_sem_threshold): small
# leading pieces collapse the ramp (the first needs just one half-width
# DVE instruction); later pieces are whole row tiles (2 MB, 16 KB
# contiguous per row) for max DMA efficiency.  Row tile 0 runs 9 DVE
# instructions (s=0 is split by in0's column halves), tiles 1-3 run 8.
PIECES = [(0, 0, 1024, 1), (0, 1024, 2048, 2), (0, 2048, 4096, 3),
          (0, 4096, 8192, 5), (0, 8192, 16384, 9),
          (1, 0, 16384, 17), (2, 0, 16384, 25), (3, 0, 16384, 33)]


def build_nc() -> bass.Bass:
    nc = bass.Bass("TRN2", target_bir_lowering=False, debug=False)
    u16, u8 = mybir.dt.uint16, mybir.dt.uint8

    # Bass's preamble memsets 4 const tensors (unused by this kernel)
    # serially on GpSimd, making it the last engine into the start
    # barrier; moving two onto DVE balances the preamble and releases
    # the barrier ~0.4us sooner (A/B-verified on HW).
    pre = nc.main_func.blocks[0]
    consts = [i for i in pre.instructions
              if i.concise().startswith(" PL Memset")]
    for i in consts[:2]:
        i.engine = mybir.EngineType.DVE

    xb = nc.declare_dram_parameter("xb", [ROWS, W], u16, isOutput=False)
    out = nc.declare_dram_parameter("out", [ROWS, OB], u8, isOutput=True)
    xb_ap, out_ap = xb.ap(), out.ap()

    from contextlib import ExitStack
    with ExitStack() as ctx:
        xt = [ctx.enter_context(nc.sbuf_tensor(f"xt{b}", [P, W], u16))
              for b in range(NRT)]
        ot = [ctx.enter_context(nc.sbuf_tensor(f"ot{b}", [P, OB], u8))
              for b in range(NRT)]

        in_sem = [ctx.enter_context(nc.semaphore(f"in_sem{b}"))
                  for b in range(NRT)]
        in0b_sem = ctx.enter_context(nc.semaphore("in0b_sem"))
        ts_sem = ctx.enter_context(nc.semaphore("ts_sem"))
        od_sem = ctx.enter_context(nc.semaphore("od_sem"))

        ctx.enter_context(nc.Block())
        block = nc.cur_block

        @block.scalar
        def _(sc: bass.BassEngine):
            # in0 is column-split across the sync+scalar rings: each half
            # spans all 128 partitions, so each DMA reaches all 16 SBUF
            # ports and the two halves stream concurrently (~2x faster
            # first-tile load than row-halves, which hit 8 ports each).
            sc.dma_start(
                xt[0][:, W // 2:W], xb_ap[0:P, W // 2:W]
            ).then_inc(in0b_sem, 16)
            for rt in range(1, NRT):
                sc.dma_start(
                    xt[rt][:], xb_ap[rt * P:(rt + 1) * P, :]
                ).then_inc(in_sem[rt], 16)

        @block.vector
        def _(vec: bass.BassEngine):
            def ts(rt, s, lo_w, hi_w):
                o = ot[rt][:, s * 2 * W + 2 * lo_w:
                           s * 2 * W + 2 * hi_w].bitcast(u16)
                vec.tensor_scalar(
                    o, xt[rt][:, lo_w:hi_w], s, 0x0101,
                    mybir.AluOpType.logical_shift_right,
                    mybir.AluOpType.bitwise_and,
                ).then_inc(ts_sem)

            # rt0: s=0 split by in0's column halves so the first piece
            # only waits for the sync-ring half of the input
            vec.wait_ge(in_sem[0], 16)
            ts(0, 0, 0, W // 2)
            vec.wait_ge(in0b_sem, 16)
            ts(0, 0, W // 2, W)
            for s in range(1, NS):
                ts(0, s, 0, W)
            for rt in range(1, NRT):
                vec.wait_ge(in_sem[rt], 16)
                for s in range(NS):
                    ts(rt, s, 0, W)

        @block.sync
        def _(sp: bass.BassEngine):
            sp.dma_start(
                xt[0][:, 0:W // 2], xb_ap[0:P, 0:W // 2]
            ).then_inc(in_sem[0], 16)
            for rt, lo_b, hi_b, thr in PIECES:
                sp.wait_ge(ts_sem, thr)
                sp.dma_start(
                    out_ap[rt * P:(rt + 1) * P, lo_b:hi_b],
                    ot[rt][:, lo_b:hi_b],
                ).then_inc(od_sem, 16)

    return nc


_NC_CACHE = None


def _get_nc():
    global _NC_CACHE
    if _NC_CACHE is None:
        _NC_CACHE = build_nc()
    return _NC_CACHE


def pack_shard(x_shard: np.ndarray) -> np.ndarray:
    """[ROWS, F] f32 -> [ROWS, W] u16: sign-normalized bitcast bytes,
    de-interleaved MSB-block-first, viewed as u16 feature pairs."""
    x_shard = np.ascontiguousarray(x_shard)
    xi = x_shard.view(np.uint32)
    xi = (xi & np.uint32(0x7FFFFFFF)) | \
        ((x_shard < 0).astype(np.uint32) << np.uint32(31))
    b = xi.view(np.uint8).reshape(x_shard.shape[0], F, 4)
    # block g holds byte (3-g): MSB block first
    xb = np.ascontiguousarray(b[:, :, ::-1].transpose(0, 2, 1))
    return xb.reshape(x_shard.shape[0], 2 * W).view(np.uint16)


def unshard(raw: np.ndarray) -> np.ndarray:
    """[ROWS, OB] u8 device planes -> [ROWS, F, K] f32."""
    arr = raw.reshape(ROWS, NS, NG, F)
    # arr[r, s, g, f] = channel 8g + (7-s); flip s then lay out (g, s')
    rev = arr[:, ::-1, :, :]
    return rev.transpose(0, 3, 2, 1).reshape(ROWS, F, K).astype(np.float32)


def make_in_maps(x: np.ndarray) -> list:
    return [{"xb": pack_shard(x[i * ROWS:(i + 1) * ROWS])}
            for i in range(N_CORES)]


def kernel(x: np.ndarray) -> np.ndarray:
    from concourse.bass_utils import run_bass_kernel_spmd

    x = np.asarray(x, dtype=np.float32)
    assert x.shape == (ROWS_TOTAL, F), x.shape
    nc = _get_nc()
    res = run_bass_kernel_spmd(nc, make_in_maps(x), list(range(N_CORES)))
    parts = [unshard(res.results[i]["out"]) for i in range(N_CORES)]
    return np.concatenate(parts, axis=0)
